# revision 14
# baseline (speedup 1.0000x reference)
"""BoxRenderLoss Trainium2 kernel (host-prepped operands, 3-op DVE combine).

loss = mean over (box, fragment) pairs of masked min-squared-distance between
each box's 10x10 fragment grid and the other box's 100-point sampled boundary,
both directions, / (2*B*FP).

Closed form: the min over the 100 boundary points decomposes into the 4 box
edges; each edge's 25-point uniform grid min is k* = clamp(round(u/s), 0, 24),
val = u - s*k*.  Per (row, i, j):
  dmin = min( ex_i + vqy_j,  ey_j + vqx_i )   where  ex = min(ux^2, vx^2),
  mask = min(mx_i, my_j) < 0                         vq = val^2,
  contribution = dmin * mask                         m  = min(u, v)

All per-row quantities are 10-wide per coordinate, O(B*10) work — they are
precomputed on the host (like the baseline's w/d/tw/ri/ss prep) and shipped
as bf16 operand tables.  The mask is folded into the min via an additive
encoding: z = BIG if outside-on-that-coord else 0, so

  contribution = min( ex_i + vqy_j,  ey_j + vqx_i,  zx_i + zy_j )

(inside => zx+zy = 0 and the two edge terms are >= 0, so the min is 0;
outside => zx+zy >= BIG and the min is dmin).  This removes the separate
mask compare+multiply op.

The device does the O(B*FP) cross-combine in 3 DVE instructions:
  1. one broadcast-AP tensor_tensor ADD builds all three expanded slabs
     (e1 | e2 | zz) in a single bf16 2x-mode pass ([128, 2400]),
  2. tensor_tensor MIN of the e1/e2 slabs,
  3. a fused scalar_tensor_tensor ((T1 mult 1.0) min zz) with accum_out
     giving per-partition partial sums [128,1] f32.
A K=128 matmul against a memset ONES column collapses partitions to [1,1]
(a [128,1] DMA-out costs ~45ns/descriptor in completion latency — ~5.7us —
so the single-descriptor out matters), then PSUM->SBUF copy and one DMA
out; the host sums 8 scalars / (2*B*FP).

Expanded layout is (h, a, b, s) with slot s innermost (2x perf mode needs
step-1 last dims on every operand): IN0 stored (h, a, s) merges (h, a);
IN1 stored (h, b, s) merges (b, s); every operand lowers to <= 3 free dims
(HW AP limit) and slabs of F are contiguous [128, 800] slices.  Partitions
carry 128 row-groups, each holding 8 rows (row r = p*8 + s).

The input table is DMA'd as two column halves on the two HWDGE queues
(sync + scalar) so the ~8ns/descriptor generation (128 descriptors each)
and the transfers run in parallel.

Hardware notes (measured):
 - tensor_tensor_reduce crashes the exec unit (NRT_EXEC_UNIT_UNRECOVERABLE)
   despite passing CoreSim; the scalar_tensor_tensor+accum_out form of the
   same fusion works.
 - GpSimd co-processing loses: a [128,800] Pool add takes ~2.1us and SBUF
   contention slows concurrent DVE ops ~3x.
 - GpSimd-issued (SWDGE) input DMA starts later than HWDGE, not earlier.
"""

import os
import numpy as np

# Exact float32 values of jnp.linspace(0.0, 1.0, 10) (fragment grid).
_LIN10 = np.array(
    [0, 1038323257, 1046711865, 1051372203, 1055100473,
     1057896676, 1059760811, 1061624946, 1063489081, 1065353216],
    dtype=np.uint32,
).view(np.float32)

_B = 4096
_FP = 100
_N_CORES = 8
_BOX_PER_CORE = _B // _N_CORES          # 512
_ROWS = 2 * _BOX_PER_CORE               # 1024 virtual rows per core
_P = 128                                # partitions
_S = _ROWS // _P                        # 8 rows (slots) per partition
_H = 3                                  # slabs: e1 | e2 | zz
_BIG = np.float32(1e30)

LAST_RESULTS = None  # BassKernelResults of the most recent run (for test.py)

_compiled = {}


def _build_nc():
    import concourse.bass as bass  # noqa: F401  (side-effect import order)
    import concourse.bacc as bacc
    import concourse.tile as tile
    from concourse import mybir

    f32 = mybir.dt.float32
    bf16 = mybir.dt.bfloat16
    Op = mybir.AluOpType

    nc = bacc.Bacc("TRN2", target_bir_lowering=False, debug=False,
                   num_devices=_N_CORES)

    # inp: per partition bf16 operand tables:
    #   cols [0 : 240)     IN0, (h, a, s) order = [ex | vqx | zx]
    #   cols [240 : 480)   IN1, (h, b, s) order = [vqy | ey | zy]
    # so F[h=0][a,b] = ex_a + vqy_b = e1, F[h=1][a,b] = vqx_a + ey_b = e2,
    # F[h=2][a,b] = zx_a + zy_b = zz.
    in_d = nc.dram_tensor("inp", [_P, 160 * _H], bf16,
                          kind="ExternalInput").ap()
    out_d = nc.dram_tensor("out", [1, 1], f32, kind="ExternalOutput").ap()

    XAB = [_P, _H, 10, 10, _S]   # expanded (h, a, b, slot) view

    with tile.TileContext(nc) as tc:
        with (
            tc.tile_pool(name="const", bufs=1) as const,
            tc.tile_pool(name="ps", bufs=1, space="PSUM") as ps,
        ):
            IN = const.tile([_P, 160 * _H], bf16)
            F = const.tile([_P, 800 * _H], bf16)
            T1 = const.tile([_P, 800], bf16)
            TJ = const.tile([_P, 800], bf16)
            ONES = const.tile([_P, 1], f32)
            part = const.tile([_P, 1], f32)
            outsb = const.tile([1, 1], f32)

            # Input halves on both HWDGE queues: descriptor generation
            # (~8ns/descriptor, 128 descriptors each) runs in parallel.
            half = 80 * _H
            nc.sync.dma_start(IN[:, 0:half], in_d[:, 0:half])
            nc.scalar.dma_start(IN[:, half:2 * half],
                                in_d[:, half:2 * half])
            # Ones column for the partition-collapse matmul (no DMA needed).
            nc.gpsimd.memset(ONES[:], 1.0)

            in0 = (IN[:, 0:half]
                   .rearrange("p (h a s) -> p h a s", h=_H, a=10)
                   .unsqueeze(3).broadcast_to(XAB))
            in1 = (IN[:, half:2 * half]
                   .rearrange("p (h b s) -> p h b s", h=_H, b=10)
                   .unsqueeze(2).broadcast_to(XAB))
            xe = F[:].rearrange("p (h a b s) -> p h a b s", h=_H, a=10, b=10)

            # 1. One DVE instruction builds all three slabs (bf16 2x mode).
            nc.vector.tensor_tensor(xe, in0, in1, Op.add)
            # 2. T1 = min(e1, e2)
            nc.vector.tensor_tensor(T1[:], F[:, 0:800], F[:, 800:1600],
                                    Op.min)
            # 3. TJ = (T1 * 1.0) min zz; part[p] = sum TJ[p, :]
            nc.vector.scalar_tensor_tensor(TJ[:], T1[:], 1.0,
                                           F[:, 1600:2400],
                                           Op.mult, Op.min,
                                           accum_out=part[:])

            # Partition collapse to [1,1]: single-descriptor DMA out.
            pr = ps.tile([1, 1], f32)
            nc.tensor.matmul(pr[:], ONES[:], part[:])
            nc.vector.tensor_copy(outsb[:], pr[:])
            nc.sync.dma_start(out_d, outsb[:])
    nc.compile()
    return nc


def _operand_tables(boxes, targets):
    """Per-row operand tables for ALL rows (both directions), float32.

    Returns ex, vq, z dicts keyed 'x'/'y', each [2, B, 10]: index 0 is the
    boxes->targets direction, index 1 the reverse.
    """
    g = _LIN10.astype(np.float64)
    out = {name: {} for name in ("ex", "vq", "z")}
    for sfx in ("x", "y"):
        for name in ("ex", "vq", "z"):
            out[name][sfx] = np.empty((2, _B, 10), np.float32)
    for di, (A, T) in enumerate(((boxes, targets), (targets, boxes))):
        A = A.astype(np.float64, copy=False)
        T = T.astype(np.float64, copy=False)
        for axis, sfx in ((0, "x"), (1, "y")):
            w = A[:, 2 + axis] - A[:, 0 + axis]
            d = A[:, 0 + axis] - T[:, 0 + axis]
            tw = T[:, 2 + axis] - T[:, 0 + axis]
            u = g[None, :] * w[:, None] + d[:, None]          # [B, 10]
            v = tw[:, None] - u
            ex = np.minimum(u * u, v * v)
            with np.errstate(divide="ignore", invalid="ignore"):
                t = np.where(tw[:, None] != 0, u * (24.0 / tw[:, None]), 0.0)
            k = np.clip(np.rint(np.maximum(t, 0.0)), 0.0, 24.0)
            val = u - k * (tw[:, None] / 24.0)
            vq = val * val
            m = np.minimum(u, v)
            z = np.where(m < 0, _BIG, np.float32(0.0))
            out["ex"][sfx][di] = ex
            out["vq"][sfx][di] = vq
            out["z"][sfx][di] = z
    return out


def _rows_to_tile(arrs):
    """Stack [nh][1024, 10] f32 arrays into the [128, nh*10*8] (h, g, s)
    bf16 tile layout (rows r = p*8 + s)."""
    import ml_dtypes
    a = np.stack(arrs, axis=1)                   # [1024, H, 10]
    a = a.reshape(_P, _S, len(arrs), 10)         # [p, s, h, g]
    a = a.transpose(0, 2, 3, 1)                  # [p, h, g, s]
    return np.ascontiguousarray(
        a.reshape(_P, -1).astype(ml_dtypes.bfloat16))


def _inputs_for_core(tabs, c):
    """Build the input map for core c from the full operand tables."""
    rows = slice(c * _BOX_PER_CORE, (c + 1) * _BOX_PER_CORE)

    def cat(d, sfx):
        return np.concatenate([d[sfx][0][rows], d[sfx][1][rows]], axis=0)

    in0 = _rows_to_tile([cat(tabs["ex"], "x"), cat(tabs["vq"], "x"),
                         cat(tabs["z"], "x")])
    in1 = _rows_to_tile([cat(tabs["vq"], "y"), cat(tabs["ex"], "y"),
                         cat(tabs["z"], "y")])
    return {"inp": np.concatenate([in0, in1], axis=1)}


def kernel(boxes: np.ndarray, targets: np.ndarray) -> np.ndarray:
    from concourse.bass_utils import run_bass_kernel_spmd

    global LAST_RESULTS
    boxes = np.ascontiguousarray(boxes, dtype=np.float32)
    targets = np.ascontiguousarray(targets, dtype=np.float32)
    assert boxes.shape == (_B, 4) and targets.shape == (_B, 4)

    if "nc" not in _compiled:
        _compiled["nc"] = _build_nc()
    nc = _compiled["nc"]

    tabs = _operand_tables(boxes, targets)
    in_maps = [_inputs_for_core(tabs, c) for c in range(_N_CORES)]

    trace = bool(int(os.environ.get("BOXLOSS_TRACE", "0")))
    res = run_bass_kernel_spmd(nc, in_maps, list(range(_N_CORES)),
                               trace=trace)
    LAST_RESULTS = res

    total = np.float64(0.0)
    for r in res.results:
        total += np.float64(r["out"].astype(np.float64).sum())
    loss = total / (2.0 * _B * _FP)
    return np.array(loss, dtype=np.float32)


# revision 18
# speedup vs baseline: 1.0827x; 1.0827x over previous
"""BoxRenderLoss Trainium2 kernel (host-prepped operands, 3-op DVE combine).

loss = mean over (box, fragment) pairs of masked min-squared-distance between
each box's 10x10 fragment grid and the other box's 100-point sampled boundary,
both directions, / (2*B*FP).

Closed form: the min over the 100 boundary points decomposes into the 4 box
edges; each edge's 25-point uniform grid min is k* = clamp(round(u/s), 0, 24),
val = u - s*k*.  Per (row, i, j):
  dmin = min( ex_i + vqy_j,  ey_j + vqx_i )   where  ex = min(ux^2, vx^2),
  mask = min(mx_i, my_j) < 0                         vq = val^2,
  contribution = dmin * mask                         m  = min(u, v)

All per-row quantities are 10-wide per coordinate, O(B*10) work — they are
precomputed on the host (like the baseline's w/d/tw/ri/ss prep) and shipped
as bf16 operand tables.  The mask is folded into the min via an additive
encoding: z = BIG if outside-on-that-coord else 0, so

  contribution = min( ex_i + vqy_j,  ey_j + vqx_i,  zx_i + zy_j )

(inside => zx+zy = 0 and the two edge terms are >= 0, so the min is 0;
outside => zx+zy >= BIG and the min is dmin).  This removes the separate
mask compare+multiply op.

The device does the O(B*FP) cross-combine in 3 DVE instructions:
  1. one broadcast-AP tensor_tensor ADD builds all three expanded slabs
     (e1 | e2 | zz) in a single bf16 2x-mode pass ([128, 2400]),
  2. tensor_tensor MIN of the e1/e2 slabs,
  3. a fused scalar_tensor_tensor ((T1 mult 1.0) min zz) with accum_out
     giving per-partition partial sums [128,1] f32.
A K=128 matmul against a memset ONES column collapses partitions to [1,1]
(a [128,1] DMA-out costs ~45ns/descriptor in completion latency — ~5.7us —
so the single-descriptor out matters), then PSUM->SBUF copy and one DMA
out; the host sums 8 scalars / (2*B*FP).

Expanded layout is (h, a, b, s) with slot s innermost (2x perf mode needs
step-1 last dims on every operand): IN0 stored (h, a, s) merges (h, a);
IN1 stored (h, b, s) merges (b, s); every operand lowers to <= 3 free dims
(HW AP limit) and slabs of F are contiguous [128, 800] slices.  Partitions
carry 128 row-groups, each holding 8 rows (row r = p*8 + s).

The input table is DMA'd as two column halves on the two HWDGE queues
(sync + scalar) so the ~8ns/descriptor generation (128 descriptors each)
and the transfers run in parallel.

Hardware notes (measured):
 - tensor_tensor_reduce crashes the exec unit (NRT_EXEC_UNIT_UNRECOVERABLE)
   despite passing CoreSim; the scalar_tensor_tensor+accum_out form of the
   same fusion works.
 - GpSimd co-processing loses: a [128,800] Pool add takes ~2.1us and SBUF
   contention slows concurrent DVE ops ~3x.
 - GpSimd-issued (SWDGE) input DMA starts later than HWDGE, not earlier.
"""

import os
import numpy as np

# Exact float32 values of jnp.linspace(0.0, 1.0, 10) (fragment grid).
_LIN10 = np.array(
    [0, 1038323257, 1046711865, 1051372203, 1055100473,
     1057896676, 1059760811, 1061624946, 1063489081, 1065353216],
    dtype=np.uint32,
).view(np.float32)

_B = 4096
_FP = 100
_N_CORES = 8
_BOX_PER_CORE = _B // _N_CORES          # 512
_ROWS = 2 * _BOX_PER_CORE               # 1024 virtual rows per core
_P = 128                                # partitions
_S = _ROWS // _P                        # 8 rows (slots) per partition
_H = 3                                  # slabs: e1 | e2 | zz
_BIG = np.float32(1e30)

# Partition-collapse mode: "matmul" (K=128 matmul vs ONES -> [1,1] out) or
# "transpose" (DVE 32x32 stream transpose + reduce -> [4,1] out, no PE).
_COLLAPSE = os.environ.get("BOXLOSS_COLLAPSE", "matmul")

LAST_RESULTS = None  # BassKernelResults of the most recent run (for test.py)

_compiled = {}


def _build_nc():
    import concourse.bass as bass  # noqa: F401  (side-effect import order)
    import concourse.bacc as bacc
    import concourse.tile as tile
    from concourse import mybir

    f32 = mybir.dt.float32
    bf16 = mybir.dt.bfloat16
    Op = mybir.AluOpType

    nc = bacc.Bacc("TRN2", target_bir_lowering=False, debug=False,
                   num_devices=_N_CORES)

    # inp: per partition bf16 operand tables:
    #   cols [0 : 240)     IN0, (h, a, s) order = [ex | vqx | zx]
    #   cols [240 : 480)   IN1, (h, b, s) order = [vqy | ey | zy]
    # so F[h=0][a,b] = ex_a + vqy_b = e1, F[h=1][a,b] = vqx_a + ey_b = e2,
    # F[h=2][a,b] = zx_a + zy_b = zz.
    in_d = nc.dram_tensor("inp", [_P, 160 * _H], bf16,
                          kind="ExternalInput").ap()
    out_shape = [1, 1] if _COLLAPSE == "matmul" else [4, 1]
    out_d = nc.dram_tensor("out", out_shape, f32, kind="ExternalOutput").ap()

    XAB = [_P, _H, 10, 10, _S]   # expanded (h, a, b, slot) view

    with tile.TileContext(nc) as tc:
        with (
            tc.tile_pool(name="const", bufs=1) as const,
            tc.tile_pool(name="ps", bufs=1, space="PSUM") as ps,
        ):
            IN = const.tile([_P, 160 * _H], bf16)
            F = const.tile([_P, 800 * _H], bf16)
            T1 = const.tile([_P, 800], bf16)
            TJ = const.tile([_P, 800], bf16)

            # Input halves on both HWDGE queues: descriptor generation
            # (~8ns/descriptor, 128 descriptors each) runs in parallel.
            half = 80 * _H
            nc.sync.dma_start(IN[:, 0:half], in_d[:, 0:half])
            nc.scalar.dma_start(IN[:, half:2 * half],
                                in_d[:, half:2 * half])
            if _COLLAPSE == "matmul":
                ONES = const.tile([_P, 1], f32)
                part = const.tile([_P, 1], f32)
                outsb = const.tile([1, 1], f32)
                # Ones column for the collapse matmul (no DMA needed).
                nc.gpsimd.memset(ONES[:], 1.0)
                accum = part[:]
            else:
                part = const.tile([_P, 32], f32)
                TP = const.tile([_P, 32], f32)
                RED = const.tile([_P, 1], f32)
                # Zero pad columns so the block transpose rows are clean.
                nc.gpsimd.memset(part[:], 0.0)
                accum = part[:, 0:1]

            in0 = (IN[:, 0:half]
                   .rearrange("p (h a s) -> p h a s", h=_H, a=10)
                   .unsqueeze(3).broadcast_to(XAB))
            in1 = (IN[:, half:2 * half]
                   .rearrange("p (h b s) -> p h b s", h=_H, b=10)
                   .unsqueeze(2).broadcast_to(XAB))
            xe = F[:].rearrange("p (h a b s) -> p h a b s", h=_H, a=10, b=10)

            # 1. One DVE instruction builds all three slabs (bf16 2x mode).
            nc.vector.tensor_tensor(xe, in0, in1, Op.add)
            # 2. T1 = min(e1, e2)
            nc.vector.tensor_tensor(T1[:], F[:, 0:800], F[:, 800:1600],
                                    Op.min)
            # 3. TJ = (T1 * 1.0) min zz; part[p] = sum TJ[p, :]
            nc.vector.scalar_tensor_tensor(TJ[:], T1[:], 1.0,
                                           F[:, 1600:2400],
                                           Op.mult, Op.min,
                                           accum_out=accum)

            if _COLLAPSE == "matmul":
                # Partition collapse to [1,1]: single-descriptor DMA out.
                pr = ps.tile([1, 1], f32)
                nc.tensor.matmul(pr[:], ONES[:], part[:])
                nc.vector.tensor_copy(outsb[:], pr[:])
                nc.sync.dma_start(out_d, outsb[:])
            else:
                # 32x32 block transpose: partition 32i, cols 0:32 receive
                # part[32i:32i+32]; reduce X -> block sums at partitions
                # {0,32,64,96}; 4-descriptor DMA out (stays far below the
                # ~45ns/descriptor completion penalty of a [128,1] out).
                nc.vector.transpose(TP[:], part[:])
                nc.vector.tensor_reduce(RED[:], TP[:],
                                        mybir.AxisListType.X, Op.add)
                nc.sync.dma_start(out_d, RED[0:_P:32, 0:1])
    nc.compile()
    return nc


def _operand_tables(boxes, targets):
    """Per-row operand tables for ALL rows (both directions), float32.

    Returns ex, vq, z dicts keyed 'x'/'y', each [2, B, 10]: index 0 is the
    boxes->targets direction, index 1 the reverse.
    """
    g = _LIN10.astype(np.float64)
    out = {name: {} for name in ("ex", "vq", "z")}
    for sfx in ("x", "y"):
        for name in ("ex", "vq", "z"):
            out[name][sfx] = np.empty((2, _B, 10), np.float32)
    for di, (A, T) in enumerate(((boxes, targets), (targets, boxes))):
        A = A.astype(np.float64, copy=False)
        T = T.astype(np.float64, copy=False)
        for axis, sfx in ((0, "x"), (1, "y")):
            w = A[:, 2 + axis] - A[:, 0 + axis]
            d = A[:, 0 + axis] - T[:, 0 + axis]
            tw = T[:, 2 + axis] - T[:, 0 + axis]
            u = g[None, :] * w[:, None] + d[:, None]          # [B, 10]
            v = tw[:, None] - u
            ex = np.minimum(u * u, v * v)
            with np.errstate(divide="ignore", invalid="ignore"):
                t = np.where(tw[:, None] != 0, u * (24.0 / tw[:, None]), 0.0)
            k = np.clip(np.rint(np.maximum(t, 0.0)), 0.0, 24.0)
            val = u - k * (tw[:, None] / 24.0)
            vq = val * val
            m = np.minimum(u, v)
            z = np.where(m < 0, _BIG, np.float32(0.0))
            out["ex"][sfx][di] = ex
            out["vq"][sfx][di] = vq
            out["z"][sfx][di] = z
    return out


def _rows_to_tile(arrs):
    """Stack [nh][1024, 10] f32 arrays into the [128, nh*10*8] (h, g, s)
    bf16 tile layout (rows r = p*8 + s)."""
    import ml_dtypes
    a = np.stack(arrs, axis=1)                   # [1024, H, 10]
    a = a.reshape(_P, _S, len(arrs), 10)         # [p, s, h, g]
    a = a.transpose(0, 2, 3, 1)                  # [p, h, g, s]
    return np.ascontiguousarray(
        a.reshape(_P, -1).astype(ml_dtypes.bfloat16))


def _inputs_for_core(tabs, c):
    """Build the input map for core c from the full operand tables."""
    rows = slice(c * _BOX_PER_CORE, (c + 1) * _BOX_PER_CORE)

    def cat(d, sfx):
        return np.concatenate([d[sfx][0][rows], d[sfx][1][rows]], axis=0)

    in0 = _rows_to_tile([cat(tabs["ex"], "x"), cat(tabs["vq"], "x"),
                         cat(tabs["z"], "x")])
    in1 = _rows_to_tile([cat(tabs["vq"], "y"), cat(tabs["ex"], "y"),
                         cat(tabs["z"], "y")])
    return {"inp": np.concatenate([in0, in1], axis=1)}


def kernel(boxes: np.ndarray, targets: np.ndarray) -> np.ndarray:
    from concourse.bass_utils import run_bass_kernel_spmd

    global LAST_RESULTS
    boxes = np.ascontiguousarray(boxes, dtype=np.float32)
    targets = np.ascontiguousarray(targets, dtype=np.float32)
    assert boxes.shape == (_B, 4) and targets.shape == (_B, 4)

    if "nc" not in _compiled:
        _compiled["nc"] = _build_nc()
    nc = _compiled["nc"]

    tabs = _operand_tables(boxes, targets)
    in_maps = [_inputs_for_core(tabs, c) for c in range(_N_CORES)]

    trace = bool(int(os.environ.get("BOXLOSS_TRACE", "0")))
    res = run_bass_kernel_spmd(nc, in_maps, list(range(_N_CORES)),
                               trace=trace)
    LAST_RESULTS = res

    total = np.float64(0.0)
    for r in res.results:
        total += np.float64(r["out"].astype(np.float64).sum())
    loss = total / (2.0 * _B * _FP)
    return np.array(loss, dtype=np.float32)
